# revision 14
# baseline (speedup 1.0000x reference)
"""AttentionBlock (b=2, c=512, 64x64) on 8 trn2 NeuronCores.

Sharding: core i handles batch i//4, query rows (i%4)*1024..+1024 (of the
4096 flattened h*w positions). Each core receives its batch's full x with
columns rotated so its own query block sits at columns 0:1024, computes
LayerNorm + K + V for all 4096 positions (replicated inside the 4-core
batch group) and Q/attention/projection for its 1024 queries.

Math reformulation (validated to ~1e-7 against the jax reference):
  - norm_w and the LayerNorm centering are folded into the QKV weights on
    the host: W~ = (W*norm_w) - row_mean(W*norm_w); then
    qkv = rsqrt(var+eps) * (W~ @ x) + b  -- no on-device mean subtraction.
  - rsqrt(var+eps) = sqrt(C) * exp(-0.5*ln(Sx2 - Sx^2/C + C*eps)); the
    sqrt(C) constant and the attention scale C**-0.5 are folded into the
    host weights, so the device only needs ln/exp (one ACT table set).
  - scores are computed transposed, sT[nk,nq] = k^T q; softmax skips the
    max-subtraction (logits are bounded ~ +-6) and defers normalization:
    av_raw = exp(sT)^T V, sumexp via a ones-vector matmul, divide at PSUM
    eviction time. The V bias is folded into the proj bias on the host.
All matmuls run as float32r (full PE speed); K/V/Q/exp(s) are stored fp16.
"""
import sys

if "/opt/trn_rl_repo" not in sys.path:
    sys.path.insert(0, "/opt/trn_rl_repo")

import numpy as np

C = 512          # channels
N = 4096         # h*w positions
NQ = 1024        # queries per core
PC = 4           # c chunks of 128
NKC = 32         # key chunks of 128
NCH = 16         # phase-1 column chunks of 256
CH = 256         # phase-1 streaming chunk width
EPS = 1e-5

_cached_nc = None


def _build_nc():
    import concourse.bass as bass
    import concourse.tile as tile
    from concourse import bacc, mybir
    from concourse.masks import make_identity

    f32 = mybir.dt.float32
    f32r = mybir.dt.float32r
    f16 = mybir.dt.float16
    AF = mybir.ActivationFunctionType
    ALU = mybir.AluOpType

    nc = bacc.Bacc(None, target_bir_lowering=False)

    xd = nc.declare_dram_parameter("x", [C, N], f32r, isOutput=False)
    wqd = nc.declare_dram_parameter("wq", [C, C], f32r, isOutput=False)
    wkd = nc.declare_dram_parameter("wk", [C, C], f32r, isOutput=False)
    wvd = nc.declare_dram_parameter("wv", [C, C], f32r, isOutput=False)
    wpd = nc.declare_dram_parameter("wp", [C, C], f32r, isOutput=False)
    bqd = nc.declare_dram_parameter("bq", [128, PC], f32, isOutput=False)
    bkd = nc.declare_dram_parameter("bk", [128, PC], f32, isOutput=False)
    bpd = nc.declare_dram_parameter("bp", [128, PC], f32, isOutput=False)
    outd = nc.declare_dram_parameter("out", [C, NQ], f32, isOutput=True)

    xdr = xd.rearrange("(a p) n -> p a n", p=128)      # [128, 4, N]
    outr = outd.rearrange("(a p) n -> p a n", p=128)   # [128, 4, NQ]

    def r32(ap):
        return ap.bitcast(f32r)

    with tile.TileContext(nc) as tc:
        from contextlib import ExitStack

        with ExitStack() as ctx:
            consts = ctx.enter_context(tc.tile_pool(name="consts", bufs=1))
            kvq = ctx.enter_context(tc.tile_pool(name="kvq", bufs=1))

            ident = consts.tile([128, 128], f32)
            make_identity(nc, ident)
            ones_col = consts.tile([128, 1], f32r)
            nc.vector.memset(ones_col.bitcast(f32), 1.0)
            ones_h = consts.tile([128, 1], f16)
            nc.vector.memset(ones_h, 1.0)
            ones_row = consts.tile([1, 128], f32r)
            nc.vector.memset(ones_row.bitcast(f32), 1.0)
            ceps = consts.tile([128, 1], f32)
            nc.vector.memset(ceps, C * EPS)

            bq_sb = consts.tile([128, PC], f32)
            nc.gpsimd.dma_start(out=bq_sb, in_=bqd[:])
            bk_sb = consts.tile([128, PC], f32)
            nc.gpsimd.dma_start(out=bk_sb, in_=bkd[:])
            bp_sb = consts.tile([128, PC], f32)
            nc.gpsimd.dma_start(out=bp_sb, in_=bpd[:])

            wq_sb = consts.tile([128, PC, C], f32r)
            nc.gpsimd.dma_start(out=wq_sb, in_=wqd.rearrange("(a p) d -> p a d", p=128))
            wk_sb = consts.tile([128, PC, C], f32r)
            nc.gpsimd.dma_start(out=wk_sb, in_=wkd.rearrange("(a p) d -> p a d", p=128))
            wv_sb = consts.tile([128, PC, C], f32r)
            nc.gpsimd.dma_start(out=wv_sb, in_=wvd.rearrange("(a p) d -> p a d", p=128))
            wp_sb = consts.tile([128, PC, C], f32r)
            nc.gpsimd.dma_start(out=wp_sb, in_=wpd.rearrange("(a p) d -> p a d", p=128))

            k_all = kvq.tile([128, PC, N], f16)    # (c, n) layout
            v_all = kvq.tile([128, NKC, C], f16)   # (n, c) layout
            q_all = kvq.tile([128, PC, NQ], f16)   # (c, nq) layout

            # R_row: per-column rsqrt factor, exp(-0.5 ln(C*(var+eps)))
            r_row = consts.tile([1, NQ], f32r)

            # ------- phase 1: stats + x'=x*r + K/V/Q for OWN 1024 columns only.
            # K and V for the other 3 ranks of the batch group arrive via two
            # AllGather collectives (K gathered while V/Q are computed).
            NOCH = NQ // CH       # own chunks (4)
            groups = [[0, 1, 2, 3], [4, 5, 6, 7]]
            dram = ctx.enter_context(tc.tile_pool(name="dram", bufs=1, space="DRAM"))
            k_loc = dram.tile([PC, 128, NQ], f16)
            v_loc = dram.tile([NQ // 128, 128, C], f16)
            k_gat = dram.tile([4, PC, 128, NQ], f16)
            v_gat = dram.tile([4, NQ // 128, 128, C], f16)

            with ExitStack() as p1:
                stage = p1.enter_context(tc.tile_pool(name="stage", bufs=1))
                xa = p1.enter_context(tc.tile_pool(name="xa", bufs=3))
                x2p = p1.enter_context(tc.tile_pool(name="x2", bufs=2))
                xpp = p1.enter_context(tc.tile_pool(name="xp", bufs=4))
                kvsb = p1.enter_context(tc.tile_pool(name="kvsb", bufs=3))
                pstat = p1.enter_context(
                    tc.tile_pool(name="pstat", bufs=2, space=bass.MemorySpace.PSUM)
                )
                rrep = p1.enter_context(
                    tc.tile_pool(name="rrep", bufs=2, space=bass.MemorySpace.PSUM)
                )
                kvps = p1.enter_context(
                    tc.tile_pool(name="kvps", bufs=4, space=bass.MemorySpace.PSUM)
                )

                # --- stats for own chunks ---
                stg_row = stage.tile([1, NOCH * 2 * CH], f32)
                for j in range(NOCH):
                    xt = xa.tile([128, PC, CH], f32r, tag="xt", name="xt")
                    nc.sync.dma_start(
                        out=xt, in_=xdr[:, :, j * CH:(j + 1) * CH]
                    )
                    x2t = x2p.tile([128, PC, CH], f32r)
                    nc.vector.tensor_mul(x2t, xt, xt)
                    ps = pstat.tile([1, 2 * CH], f32)
                    for ci in range(PC):
                        nc.tensor.matmul(
                            ps[:, 0:CH], ones_col, xt[:, ci, :],
                            start=(ci == 0), stop=(ci == PC - 1),
                        )
                    for ci in range(PC):
                        nc.tensor.matmul(
                            ps[:, CH:2 * CH], ones_col, x2t[:, ci, :],
                            start=(ci == 0), stop=(ci == PC - 1),
                        )
                    nc.scalar.copy(
                        stg_row[0:1, j * 2 * CH:(j + 1) * 2 * CH], ps
                    )
                stg = stage.tile([NOCH, 2 * CH], f32)
                nc.sync.dma_start(out=stg, in_=stg_row)
                u2 = stage.tile([NOCH, CH], f32)
                nc.vector.tensor_mul(u2, stg[:, 0:CH], stg[:, 0:CH])
                nc.vector.tensor_scalar_mul(u2, u2, 1.0 / C)
                w1 = stage.tile([NOCH, CH], f32)
                nc.vector.tensor_tensor(
                    out=w1, in0=stg[:, CH:2 * CH], in1=u2, op=ALU.subtract
                )
                nc.scalar.activation(w1, w1, AF.Ln, bias=ceps[0:NOCH])
                rt = stage.tile([NOCH, CH], f32r)
                nc.scalar.activation(rt, w1, AF.Exp, scale=-0.5)
                nc.sync.dma_start(out=r_row, in_=rt)

                # --- x' for both own pairs ---
                xps = []  # [pair][half]
                for j2 in range(2):
                    row = []
                    for h in range(2):
                        j = 2 * j2 + h
                        xt = xa.tile([128, PC, CH], f32r, tag="xt", name="xtb")
                        nc.sync.dma_start(
                            out=xt, in_=xdr[:, :, j * CH:(j + 1) * CH]
                        )
                        rr = rrep.tile([128, CH], f32)
                        nc.tensor.matmul(
                            rr, ones_row,
                            r_row[0:1, j * CH:(j + 1) * CH],
                            start=True, stop=True,
                        )
                        xpt = xpp.tile(
                            [128, PC, CH], f32r, tag="xp", name=f"xp{j2}{h}"
                        )
                        nc.vector.tensor_mul(
                            xpt, xt, rr.unsqueeze(1).broadcast_to([128, PC, CH])
                        )
                        row.append(xpt)
                    xps.append(row)

                # --- K for own columns, then gather ---
                for j2 in range(2):
                    for co in range(PC):
                        kp = kvps.tile([128, 2 * CH], f32, tag="kvqps", name="kp")
                        for h in range(2):
                            for ci in range(PC):
                                nc.tensor.matmul(
                                    kp[:, h * CH:(h + 1) * CH],
                                    wk_sb[:, ci, co * 128:(co + 1) * 128],
                                    xps[j2][h][:, ci, :],
                                    start=(ci == 0), stop=(ci == PC - 1),
                                )
                        ks = kvsb.tile([128, 2 * CH], f16, tag="kv", name="ks")
                        nc.scalar.activation(
                            ks, kp, AF.Identity, bias=bk_sb[:, co:co + 1],
                        )
                        nc.scalar.dma_start(
                            out=k_loc[co, :, j2 * 512:(j2 + 1) * 512], in_=ks
                        )
                nc.gpsimd.collective_compute(
                    "AllGather", mybir.AluOpType.bypass,
                    replica_groups=groups, ins=[k_loc], outs=[k_gat],
                )

                # --- V for own rows, then gather ---
                for j2 in range(2):
                    for h in range(2):
                        for s in range(2):
                            vp = kvps.tile([128, C], f32, tag="kvqps", name="vp")
                            for ci in range(PC):
                                nc.tensor.matmul(
                                    vp,
                                    xps[j2][h][:, ci, s * 128:(s + 1) * 128],
                                    wv_sb[:, ci, :],
                                    start=(ci == 0), stop=(ci == PC - 1),
                                )
                            vs = kvsb.tile([128, C], f16, tag="kv", name="vs")
                            nc.vector.tensor_copy(vs, vp)
                            nc.scalar.dma_start(
                                out=v_loc[4 * j2 + 2 * h + s], in_=vs
                            )
                nc.gpsimd.collective_compute(
                    "AllGather", mybir.AluOpType.bypass,
                    replica_groups=groups, ins=[v_loc], outs=[v_gat],
                )

                # --- Q for own columns (kept local) ---
                for j2 in range(2):
                    for co in range(PC):
                        qp = kvps.tile([128, 2 * CH], f32, tag="kvqps", name="qp")
                        for h in range(2):
                            for ci in range(PC):
                                nc.tensor.matmul(
                                    qp[:, h * CH:(h + 1) * CH],
                                    wq_sb[:, ci, co * 128:(co + 1) * 128],
                                    xps[j2][h][:, ci, :],
                                    start=(ci == 0), stop=(ci == PC - 1),
                                )
                        nc.scalar.activation(
                            q_all[:, co, j2 * 512:(j2 + 1) * 512], qp,
                            AF.Identity, bias=bq_sb[:, co:co + 1],
                        )

                # --- import gathered K/V into SBUF ---
                for r in range(4):
                    nc.sync.dma_start(
                        out=k_all[:, :, r * NQ:(r + 1) * NQ],
                        in_=k_gat[r].rearrange("a p n -> p a n"),
                    )
                for r in range(4):
                    nc.sync.dma_start(
                        out=v_all[:, r * (NQ // 128):(r + 1) * (NQ // 128), :],
                        in_=v_gat[r].rearrange("s p c -> p s c"),
                    )

            # ---------------- attention + projection, per 512-query group ----------------
            with ExitStack() as pat:
                stp = pat.enter_context(
                    tc.tile_pool(name="stp", bufs=2, space=bass.MemorySpace.PSUM)
                )
                avp_pool = pat.enter_context(
                    tc.tile_pool(name="avp", bufs=4, space=bass.MemorySpace.PSUM)
                )
                sep_pool = pat.enter_context(
                    tc.tile_pool(name="sep", bufs=1, space=bass.MemorySpace.PSUM)
                )
                tp_pool = pat.enter_context(
                    tc.tile_pool(name="tp", bufs=1, space=bass.MemorySpace.PSUM)
                )
                ptp = pat.enter_context(tc.tile_pool(name="ptp", bufs=4))
                avn_pool = pat.enter_context(tc.tile_pool(name="avn", bufs=4))
                avt_pool = pat.enter_context(tc.tile_pool(name="avt", bufs=2))
                out_pool = pat.enter_context(tc.tile_pool(name="outp", bufs=2))
                xres_pool = pat.enter_context(tc.tile_pool(name="xres", bufs=1))
                small = pat.enter_context(tc.tile_pool(name="small", bufs=2))

                for g in range(2):
                    q0 = g * 512
                    avps = [avp_pool.tile([128, C], f32, tag="av", name=f"avp{s}") for s in range(4)]
                    sep = sep_pool.tile([1, 512], f32)
                    for jk in range(NKC):
                        st = stp.tile([128, 512], f32)
                        for ci in range(PC):
                            nc.tensor.matmul(
                                st,
                                k_all[:, ci, jk * 128:(jk + 1) * 128],
                                q_all[:, ci, q0:q0 + 512],
                                start=(ci == 0), stop=(ci == PC - 1),
                            )
                        pt = ptp.tile([128, 512], f16)
                        nc.scalar.activation(pt, st, AF.Exp)
                        nc.tensor.matmul(
                            sep, ones_h, pt,
                            start=(jk == 0), stop=(jk == NKC - 1),
                        )
                        for s in range(4):
                            nc.tensor.matmul(
                                avps[s],
                                pt[:, s * 128:(s + 1) * 128],
                                v_all[:, jk, :],
                                start=(jk == 0), stop=(jk == NKC - 1),
                            )

                    # 1/sumexp as per-partition columns
                    serow = small.tile([1, 512], f32)
                    nc.scalar.copy(serow, sep)
                    rc_ps = tp_pool.tile([128, 512], f32, tag="tp")
                    for s in range(4):
                        nc.tensor.transpose(
                            rc_ps[:, s:s + 1],
                            serow[0:1, s * 128:(s + 1) * 128],
                            ident[0:1, 0:1],
                        )
                    rc_sb = small.tile([128, 4], f32)
                    nc.vector.reciprocal(rc_sb, rc_ps[:, 0:4])

                    # normalize + transpose to (c, nq)
                    avns = []
                    for s in range(4):
                        avn = avn_pool.tile([128, C], f32, tag="avn", name=f"avn{s}")
                        nc.vector.tensor_scalar_mul(avn, avps[s], rc_sb[:, s:s + 1])
                        avns.append(avn)
                    avt = avt_pool.tile([128, PC, 512], f32r)
                    for ci in range(PC):
                        tps = tp_pool.tile([128, 512], f32, tag="tp")
                        for s in range(4):
                            nc.tensor.transpose(
                                tps[:, s * 128:(s + 1) * 128],
                                avns[s][:, ci * 128:(ci + 1) * 128],
                                ident,
                            )
                        nc.vector.tensor_copy(avt[:, ci, :], tps)

                    # projection + bias + residual + store
                    out_t = out_pool.tile([128, PC, 512], f32)
                    for co in range(PC):
                        pop = tp_pool.tile([128, 512], f32, tag="tp")
                        for ci in range(PC):
                            nc.tensor.matmul(
                                pop,
                                wp_sb[:, ci, co * 128:(co + 1) * 128],
                                avt[:, ci, :],
                                start=(ci == 0), stop=(ci == PC - 1),
                            )
                        nc.scalar.activation(
                            out_t[:, co, :], pop,
                            AF.Identity, bias=bp_sb[:, co:co + 1],
                        )
                    xres = xres_pool.tile([128, PC, 512], f32r)
                    nc.sync.dma_start(out=xres, in_=xdr[:, :, q0:q0 + 512])
                    nc.vector.tensor_add(out_t, out_t, xres)
                    nc.sync.dma_start(
                        out=outr[:, :, q0:q0 + 512], in_=out_t
                    )

    nc.compile()
    return nc


def _get_nc():
    global _cached_nc
    if _cached_nc is None:
        _cached_nc = _build_nc()
    return _cached_nc


def kernel(x, norm_w, w_qkv, b_qkv, w_proj, b_proj):
    x = np.asarray(x, dtype=np.float32)
    norm_w = np.asarray(norm_w, dtype=np.float32)
    w_qkv = np.asarray(w_qkv, dtype=np.float32)
    b_qkv = np.asarray(b_qkv, dtype=np.float32)
    w_proj = np.asarray(w_proj, dtype=np.float32)
    b_proj = np.asarray(b_proj, dtype=np.float32)

    B = x.shape[0]
    scale = C ** -0.5
    sc = np.sqrt(C).astype(np.float32)

    # fold norm_w + LN centering + sqrt(C) (+ attention scale for q) into weights
    Wq = w_qkv[0:C] * norm_w[None, :]
    Wk = w_qkv[C:2 * C] * norm_w[None, :]
    Wv = w_qkv[2 * C:3 * C] * norm_w[None, :]
    Wqt = np.ascontiguousarray(((Wq - Wq.mean(1, keepdims=True)) * (sc * scale)).T)
    Wkt = np.ascontiguousarray(((Wk - Wk.mean(1, keepdims=True)) * sc).T)
    Wvt = np.ascontiguousarray(((Wv - Wv.mean(1, keepdims=True)) * sc).T)
    Wpt = np.ascontiguousarray(w_proj.T)

    def cols(b):  # [C] -> [128, 4] chunk-column layout
        return np.ascontiguousarray(b.reshape(PC, 128).T)

    bq = cols(b_qkv[0:C] * scale)
    bk = cols(b_qkv[C:2 * C])
    bv = b_qkv[2 * C:3 * C]
    bpt = cols(b_proj + w_proj @ bv)

    in_maps = []
    for core in range(8):
        bi, qi = core // 4, core % 4
        xl = np.ascontiguousarray(
            np.roll(x[bi].reshape(C, N), -qi * NQ, axis=1)
        )
        in_maps.append({
            "x": xl, "wq": Wqt, "wk": Wkt, "wv": Wvt, "wp": Wpt,
            "bq": bq, "bk": bk, "bp": bpt,
        })

    from concourse.bass_utils import run_bass_kernel_spmd

    nc = _get_nc()
    res = run_bass_kernel_spmd(nc, in_maps, core_ids=list(range(8)))

    out = np.empty((B, C, N), dtype=np.float32)
    for core in range(8):
        bi, qi = core // 4, core % 4
        out[bi][:, qi * NQ:(qi + 1) * NQ] = res.results[core]["out"]
    return out.reshape(x.shape)


# revision 17
# speedup vs baseline: 1.2770x; 1.2770x over previous
"""AttentionBlock (b=2, c=512, 64x64) on 8 trn2 NeuronCores.

Sharding: core i handles batch i//4, query rows (i%4)*1024..+1024 (of the
4096 flattened h*w positions). Each core receives its batch's full x with
columns rotated so its own query block sits at columns 0:1024, computes
LayerNorm + K + V for all 4096 positions (replicated inside the 4-core
batch group) and Q/attention/projection for its 1024 queries.

Math reformulation (validated to ~1e-7 against the jax reference):
  - norm_w and the LayerNorm centering are folded into the QKV weights on
    the host: W~ = (W*norm_w) - row_mean(W*norm_w); then
    qkv = rsqrt(var+eps) * (W~ @ x) + b  -- no on-device mean subtraction.
  - rsqrt(var+eps) = sqrt(C) * exp(-0.5*ln(Sx2 - Sx^2/C + C*eps)); the
    sqrt(C) constant and the attention scale C**-0.5 are folded into the
    host weights, so the device only needs ln/exp (one ACT table set).
  - scores are computed transposed, sT[nk,nq] = k^T q; softmax skips the
    max-subtraction (logits are bounded ~ +-6) and defers normalization:
    av_raw = exp(sT)^T V, sumexp via a ones-vector matmul, divide at PSUM
    eviction time. The V bias is folded into the proj bias on the host.
All matmuls run as float32r (full PE speed); K/V/Q/exp(s) are stored fp16.
"""
import sys

if "/opt/trn_rl_repo" not in sys.path:
    sys.path.insert(0, "/opt/trn_rl_repo")

import numpy as np

C = 512          # channels
N = 4096         # h*w positions
NQ = 1024        # queries per core
PC = 4           # c chunks of 128
NKC = 32         # key chunks of 128
NCH = 16         # phase-1 column chunks of 256
CH = 256         # phase-1 streaming chunk width
EPS = 1e-5

_cached_nc = None


def _build_nc():
    import concourse.bass as bass
    import concourse.tile as tile
    from concourse import bacc, mybir
    from concourse.masks import make_identity

    f32 = mybir.dt.float32
    f32r = mybir.dt.float32r
    f16 = mybir.dt.float16
    AF = mybir.ActivationFunctionType
    ALU = mybir.AluOpType

    nc = bacc.Bacc(None, target_bir_lowering=False)

    xd = nc.declare_dram_parameter("x", [C, N], f32r, isOutput=False)
    wqd = nc.declare_dram_parameter("wq", [C, C], f32r, isOutput=False)
    wkd = nc.declare_dram_parameter("wk", [C, C], f32r, isOutput=False)
    wvd = nc.declare_dram_parameter("wv", [C, C], f32r, isOutput=False)
    wpd = nc.declare_dram_parameter("wp", [C, C], f32r, isOutput=False)
    bqd = nc.declare_dram_parameter("bq", [128, PC], f32, isOutput=False)
    bkd = nc.declare_dram_parameter("bk", [128, PC], f32, isOutput=False)
    bpd = nc.declare_dram_parameter("bp", [128, PC], f32, isOutput=False)
    outd = nc.declare_dram_parameter("out", [C, NQ], f32, isOutput=True)

    xdr = xd.rearrange("(a p) n -> p a n", p=128)      # [128, 4, N]
    outr = outd.rearrange("(a p) n -> p a n", p=128)   # [128, 4, NQ]

    def r32(ap):
        return ap.bitcast(f32r)

    with tile.TileContext(nc) as tc:
        from contextlib import ExitStack

        with ExitStack() as ctx:
            consts = ctx.enter_context(tc.tile_pool(name="consts", bufs=1))
            kvq = ctx.enter_context(tc.tile_pool(name="kvq", bufs=1))

            ident = consts.tile([128, 128], f32)
            make_identity(nc, ident)
            ones_col = consts.tile([128, 1], f32r)
            nc.vector.memset(ones_col.bitcast(f32), 1.0)
            ones_row = consts.tile([1, 128], f32r)
            nc.vector.memset(ones_row.bitcast(f32), 1.0)
            ceps = consts.tile([128, 1], f32)
            nc.vector.memset(ceps, C * EPS)

            bq_sb = consts.tile([128, PC], f32)
            nc.gpsimd.dma_start(out=bq_sb, in_=bqd[:])
            bk_sb = consts.tile([128, PC], f32)
            nc.gpsimd.dma_start(out=bk_sb, in_=bkd[:])
            bp_sb = consts.tile([128, PC], f32)
            nc.gpsimd.dma_start(out=bp_sb, in_=bpd[:])

            wq_sb = consts.tile([128, PC, C], f32r)
            nc.gpsimd.dma_start(out=wq_sb, in_=wqd.rearrange("(a p) d -> p a d", p=128))
            wk_sb = consts.tile([128, PC, C], f32r)
            nc.gpsimd.dma_start(out=wk_sb, in_=wkd.rearrange("(a p) d -> p a d", p=128))
            wv_sb = consts.tile([128, PC, C], f32r)
            nc.gpsimd.dma_start(out=wv_sb, in_=wvd.rearrange("(a p) d -> p a d", p=128))
            wp_sb = consts.tile([128, PC, C], f32r)
            nc.gpsimd.dma_start(out=wp_sb, in_=wpd.rearrange("(a p) d -> p a d", p=128))

            k_all = kvq.tile([128, PC, N], f16)    # (c, n) layout
            v_all = kvq.tile([128, NKC, C], f16)   # (n, c) layout
            q_all = kvq.tile([128, PC, NQ], f16)   # (c, nq) layout

            # R_row: per-column rsqrt factor, exp(-0.5 ln(C*(var+eps)))
            r_row = consts.tile([1, N], f32r)

            # ------- phase 1: two passes. A: column stats (all chunks, one
            # Ln+Exp batch = one table set load). B: x'=x*r + K/V/Q matmuls.
            with ExitStack() as p1:
                stage = p1.enter_context(tc.tile_pool(name="stage", bufs=2))
                xa = p1.enter_context(tc.tile_pool(name="xa", bufs=3))
                x2p = p1.enter_context(tc.tile_pool(name="x2", bufs=2))
                xb = p1.enter_context(tc.tile_pool(name="xb", bufs=3))
                xpp = p1.enter_context(tc.tile_pool(name="xp", bufs=3))
                pstat = p1.enter_context(
                    tc.tile_pool(name="pstat", bufs=2, space=bass.MemorySpace.PSUM)
                )
                rrep = p1.enter_context(
                    tc.tile_pool(name="rrep", bufs=2, space=bass.MemorySpace.PSUM)
                )
                kvps = p1.enter_context(
                    tc.tile_pool(name="kvps", bufs=4, space=bass.MemorySpace.PSUM)
                )

                # --- pass A: stats, in halves so half-0's math chain overlaps
                # half-1's stats matmuls; x loads rotate across DMA queues ---
                HN = NCH // 2
                dmaeng = [nc.sync, nc.scalar, nc.gpsimd]
                for hh in range(2):
                    stg_row = stage.tile([1, HN * 2 * CH], f32, name="stg_row")
                    for jj in range(HN):
                        j = hh * HN + jj
                        xt = xa.tile([128, PC, CH], f32r, tag="xt", name="xt")
                        dmaeng[j % 3].dma_start(
                            out=xt, in_=xdr[:, :, j * CH:(j + 1) * CH]
                        )
                        x2t = x2p.tile([128, PC, CH], f32r)
                        nc.vector.tensor_mul(x2t, xt, xt)
                        ps = pstat.tile([1, 2 * CH], f32)
                        for ci in range(PC):
                            nc.tensor.matmul(
                                ps[:, 0:CH], ones_col, xt[:, ci, :],
                                start=(ci == 0), stop=(ci == PC - 1),
                            )
                        for ci in range(PC):
                            nc.tensor.matmul(
                                ps[:, CH:2 * CH], ones_col, x2t[:, ci, :],
                                start=(ci == 0), stop=(ci == PC - 1),
                            )
                        nc.scalar.copy(
                            stg_row[0:1, jj * 2 * CH:(jj + 1) * 2 * CH], ps
                        )
                    stg = stage.tile([HN, 2 * CH], f32, name="stg")
                    nc.sync.dma_start(out=stg, in_=stg_row)
                    u2 = stage.tile([HN, CH], f32, name="u2")
                    nc.vector.tensor_mul(u2, stg[:, 0:CH], stg[:, 0:CH])
                    nc.vector.tensor_scalar_mul(u2, u2, 1.0 / C)
                    w1 = stage.tile([HN, CH], f32, name="w1")
                    nc.vector.tensor_tensor(
                        out=w1, in0=stg[:, CH:2 * CH], in1=u2, op=ALU.subtract
                    )
                    nc.scalar.activation(w1, w1, AF.Ln, bias=ceps[0:HN])
                    rt = stage.tile([HN, CH], f32r, name="rt")
                    nc.scalar.activation(rt, w1, AF.Exp, scale=-0.5)
                    nc.sync.dma_start(
                        out=r_row[0:1, hh * HN * CH:(hh + 1) * HN * CH], in_=rt
                    )

                # --- pass B: x', K, V, Q ---
                for j2 in range(NCH // 2):  # 512-col pairs
                    xps = []
                    for h in range(2):
                        j = 2 * j2 + h
                        xt = xb.tile([128, PC, CH], f32r, name="xtb")
                        nc.gpsimd.dma_start(
                            out=xt, in_=xdr[:, :, j * CH:(j + 1) * CH]
                        )
                        rr = rrep.tile([128, CH], f32)
                        nc.tensor.matmul(
                            rr, ones_row,
                            r_row[0:1, j * CH:(j + 1) * CH],
                            start=True, stop=True,
                        )
                        xpt = xpp.tile([128, PC, CH], f32r)
                        nc.vector.tensor_mul(
                            xpt, xt, rr.unsqueeze(1).broadcast_to([128, PC, CH])
                        )
                        xps.append(xpt)

                    for co in range(PC):
                        kp = kvps.tile([128, 2 * CH], f32, tag="kvqps", name="kp")
                        for h in range(2):
                            for ci in range(PC):
                                nc.tensor.matmul(
                                    kp[:, h * CH:(h + 1) * CH],
                                    wk_sb[:, ci, co * 128:(co + 1) * 128],
                                    xps[h][:, ci, :],
                                    start=(ci == 0), stop=(ci == PC - 1),
                                )
                        nc.scalar.activation(
                            k_all[:, co, j2 * 512:(j2 + 1) * 512], kp,
                            AF.Identity, bias=bk_sb[:, co:co + 1],
                        )
                    for h in range(2):
                        for s in range(2):
                            vp = kvps.tile([128, C], f32, tag="kvqps", name="vp")
                            for ci in range(PC):
                                nc.tensor.matmul(
                                    vp,
                                    xps[h][:, ci, s * 128:(s + 1) * 128],
                                    wv_sb[:, ci, :],
                                    start=(ci == 0), stop=(ci == PC - 1),
                                )
                            nc.vector.tensor_copy(
                                v_all[:, 4 * j2 + 2 * h + s, :], vp
                            )
                    if j2 < 2:
                        for co in range(PC):
                            qp = kvps.tile([128, 2 * CH], f32, tag="kvqps", name="qp")
                            for h in range(2):
                                for ci in range(PC):
                                    nc.tensor.matmul(
                                        qp[:, h * CH:(h + 1) * CH],
                                        wq_sb[:, ci, co * 128:(co + 1) * 128],
                                        xps[h][:, ci, :],
                                        start=(ci == 0), stop=(ci == PC - 1),
                                    )
                            nc.scalar.activation(
                                q_all[:, co, j2 * 512:(j2 + 1) * 512], qp,
                                AF.Identity, bias=bq_sb[:, co:co + 1],
                            )

            # ---------------- attention + projection, per 512-query group ----------------
            with ExitStack() as pat:
                stp = pat.enter_context(
                    tc.tile_pool(name="stp", bufs=3, space=bass.MemorySpace.PSUM)
                )
                avp_pool = pat.enter_context(
                    tc.tile_pool(name="avp", bufs=4, space=bass.MemorySpace.PSUM)
                )
                tp_pool = pat.enter_context(
                    tc.tile_pool(name="tp", bufs=1, space=bass.MemorySpace.PSUM)
                )
                ptp = pat.enter_context(tc.tile_pool(name="ptp", bufs=4))
                avn_pool = pat.enter_context(tc.tile_pool(name="avn", bufs=4))
                avt_pool = pat.enter_context(tc.tile_pool(name="avt", bufs=2))
                out_pool = pat.enter_context(tc.tile_pool(name="outp", bufs=2))
                xres_pool = pat.enter_context(tc.tile_pool(name="xres", bufs=2))
                small = pat.enter_context(tc.tile_pool(name="small", bufs=2))

                # prefetch residual inputs for both groups
                xres_ts = []
                for g in range(2):
                    xres = xres_pool.tile(
                        [128, PC, 512], f32r, tag="xres", name=f"xres{g}"
                    )
                    (nc.sync if g == 0 else nc.scalar).dma_start(
                        out=xres, in_=xdr[:, :, g * 512:(g + 1) * 512]
                    )
                    xres_ts.append(xres)

                for g in range(2):
                    q0 = g * 512
                    avps = [avp_pool.tile([128, C], f32, tag="av", name=f"avp{s}") for s in range(4)]
                    # sumexp accumulated on DVE (two chains), reduced by one matmul
                    acc0 = small.tile([128, 512], f32r, tag="acc", name="acc0")
                    acc1 = small.tile([128, 512], f32r, tag="acc", name="acc1")
                    for jk in range(NKC):
                        st = stp.tile([128, 512], f32)
                        for ci in range(PC):
                            nc.tensor.matmul(
                                st,
                                k_all[:, ci, jk * 128:(jk + 1) * 128],
                                q_all[:, ci, q0:q0 + 512],
                                start=(ci == 0), stop=(ci == PC - 1),
                            )
                        pt = ptp.tile([128, 512], f16)
                        nc.scalar.activation(pt, st, AF.Exp)
                        acc = acc0 if jk % 2 == 0 else acc1
                        if jk < 2:
                            nc.vector.tensor_copy(acc, pt)
                        else:
                            nc.vector.tensor_add(acc, acc, pt)
                        for s in range(4):
                            nc.tensor.matmul(
                                avps[s],
                                pt[:, s * 128:(s + 1) * 128],
                                v_all[:, jk, :],
                                start=(jk == 0), stop=(jk == NKC - 1),
                            )
                    nc.vector.tensor_add(acc0, acc0, acc1)
                    sep = tp_pool.tile([1, 512], f32, tag="tp", name="sep")
                    nc.tensor.matmul(sep, ones_col, acc0, start=True, stop=True)

                    # 1/sumexp as per-partition columns
                    serow = small.tile([1, 512], f32)
                    nc.scalar.copy(serow, sep)
                    rc_ps = tp_pool.tile([128, 512], f32, tag="tp")
                    for s in range(4):
                        nc.tensor.transpose(
                            rc_ps[:, s:s + 1],
                            serow[0:1, s * 128:(s + 1) * 128],
                            ident[0:1, 0:1],
                        )
                    rc_sb = small.tile([128, 4], f32)
                    nc.vector.reciprocal(rc_sb, rc_ps[:, 0:4])

                    # normalize + transpose to (c, nq)
                    avns = []
                    for s in range(4):
                        avn = avn_pool.tile([128, C], f32, tag="avn", name=f"avn{s}")
                        nc.vector.tensor_scalar_mul(avn, avps[s], rc_sb[:, s:s + 1])
                        avns.append(avn)
                    avt = avt_pool.tile([128, PC, 512], f32r)
                    for ci in range(PC):
                        tps = tp_pool.tile([128, 512], f32, tag="tp")
                        for s in range(4):
                            nc.tensor.transpose(
                                tps[:, s * 128:(s + 1) * 128],
                                avns[s][:, ci * 128:(ci + 1) * 128],
                                ident,
                            )
                        nc.vector.tensor_copy(avt[:, ci, :], tps)

                    # projection + bias + residual + store
                    out_t = out_pool.tile([128, PC, 512], f32)
                    for co in range(PC):
                        pop = tp_pool.tile([128, 512], f32, tag="tp")
                        for ci in range(PC):
                            nc.tensor.matmul(
                                pop,
                                wp_sb[:, ci, co * 128:(co + 1) * 128],
                                avt[:, ci, :],
                                start=(ci == 0), stop=(ci == PC - 1),
                            )
                        nc.scalar.activation(
                            out_t[:, co, :], pop,
                            AF.Identity, bias=bp_sb[:, co:co + 1],
                        )
                        nc.vector.tensor_add(
                            out_t[:, co, :], out_t[:, co, :],
                            xres_ts[g][:, co, :],
                        )
                        nc.sync.dma_start(
                            out=outr[:, co, q0:q0 + 512], in_=out_t[:, co, :]
                        )


    nc.compile()
    return nc


def _get_nc():
    global _cached_nc
    if _cached_nc is None:
        _cached_nc = _build_nc()
    return _cached_nc


def kernel(x, norm_w, w_qkv, b_qkv, w_proj, b_proj):
    x = np.asarray(x, dtype=np.float32)
    norm_w = np.asarray(norm_w, dtype=np.float32)
    w_qkv = np.asarray(w_qkv, dtype=np.float32)
    b_qkv = np.asarray(b_qkv, dtype=np.float32)
    w_proj = np.asarray(w_proj, dtype=np.float32)
    b_proj = np.asarray(b_proj, dtype=np.float32)

    B = x.shape[0]
    scale = C ** -0.5
    sc = np.sqrt(C).astype(np.float32)

    # fold norm_w + LN centering + sqrt(C) (+ attention scale for q) into weights
    Wq = w_qkv[0:C] * norm_w[None, :]
    Wk = w_qkv[C:2 * C] * norm_w[None, :]
    Wv = w_qkv[2 * C:3 * C] * norm_w[None, :]
    Wqt = np.ascontiguousarray(((Wq - Wq.mean(1, keepdims=True)) * (sc * scale)).T)
    Wkt = np.ascontiguousarray(((Wk - Wk.mean(1, keepdims=True)) * sc).T)
    Wvt = np.ascontiguousarray(((Wv - Wv.mean(1, keepdims=True)) * sc).T)
    Wpt = np.ascontiguousarray(w_proj.T)

    def cols(b):  # [C] -> [128, 4] chunk-column layout
        return np.ascontiguousarray(b.reshape(PC, 128).T)

    bq = cols(b_qkv[0:C] * scale)
    bk = cols(b_qkv[C:2 * C])
    bv = b_qkv[2 * C:3 * C]
    bpt = cols(b_proj + w_proj @ bv)

    in_maps = []
    for core in range(8):
        bi, qi = core // 4, core % 4
        xl = np.ascontiguousarray(
            np.roll(x[bi].reshape(C, N), -qi * NQ, axis=1)
        )
        in_maps.append({
            "x": xl, "wq": Wqt, "wk": Wkt, "wv": Wvt, "wp": Wpt,
            "bq": bq, "bk": bk, "bp": bpt,
        })

    from concourse.bass_utils import run_bass_kernel_spmd

    nc = _get_nc()
    res = run_bass_kernel_spmd(nc, in_maps, core_ids=list(range(8)))

    out = np.empty((B, C, N), dtype=np.float32)
    for core in range(8):
        bi, qi = core // 4, core % 4
        out[bi][:, qi * NQ:(qi + 1) * NQ] = res.results[core]["out"]
    return out.reshape(x.shape)


# revision 18
# speedup vs baseline: 1.3118x; 1.0272x over previous
"""AttentionBlock (b=2, c=512, 64x64) on 8 trn2 NeuronCores.

Sharding: core i handles batch i//4, query rows (i%4)*1024..+1024 (of the
4096 flattened h*w positions). Each core receives its batch's full x with
columns rotated so its own query block sits at columns 0:1024, computes
LayerNorm + K + V for all 4096 positions (replicated inside the 4-core
batch group) and Q/attention/projection for its 1024 queries.

Math reformulation (validated to ~1e-7 against the jax reference):
  - norm_w and the LayerNorm centering are folded into the QKV weights on
    the host: W~ = (W*norm_w) - row_mean(W*norm_w); then
    qkv = rsqrt(var+eps) * (W~ @ x) + b  -- no on-device mean subtraction.
  - rsqrt(var+eps) = sqrt(C) * exp(-0.5*ln(Sx2 - Sx^2/C + C*eps)); the
    sqrt(C) constant and the attention scale C**-0.5 are folded into the
    host weights, so the device only needs ln/exp (one ACT table set).
  - scores are computed transposed, sT[nk,nq] = k^T q; softmax skips the
    max-subtraction (logits are bounded ~ +-6) and defers normalization:
    av_raw = exp(sT)^T V, sumexp via a ones-vector matmul, divide at PSUM
    eviction time. The V bias is folded into the proj bias on the host.
All matmuls run as float32r (full PE speed); K/V/Q/exp(s) are stored fp16.
"""
import sys

if "/opt/trn_rl_repo" not in sys.path:
    sys.path.insert(0, "/opt/trn_rl_repo")

import numpy as np

C = 512          # channels
N = 4096         # h*w positions
NQ = 1024        # queries per core
PC = 4           # c chunks of 128
NKC = 32         # key chunks of 128
NCH = 16         # phase-1 column chunks of 256
CH = 256         # phase-1 streaming chunk width
EPS = 1e-5

_cached_nc = None


def _build_nc():
    import concourse.bass as bass
    import concourse.tile as tile
    from concourse import bacc, mybir
    from concourse.masks import make_identity

    f32 = mybir.dt.float32
    f32r = mybir.dt.float32r
    f16 = mybir.dt.float16
    AF = mybir.ActivationFunctionType
    ALU = mybir.AluOpType

    nc = bacc.Bacc(None, target_bir_lowering=False)

    xd = nc.declare_dram_parameter("x", [NCH, 128, PC, CH], f32r, isOutput=False)
    wqd = nc.declare_dram_parameter("wq", [128, PC, C], f32r, isOutput=False)
    wkd = nc.declare_dram_parameter("wk", [128, PC, C], f32r, isOutput=False)
    wvd = nc.declare_dram_parameter("wv", [128, PC, C], f32r, isOutput=False)
    wpd = nc.declare_dram_parameter("wp", [128, PC, C], f32r, isOutput=False)
    bqd = nc.declare_dram_parameter("bq", [128, PC], f32, isOutput=False)
    bkd = nc.declare_dram_parameter("bk", [128, PC], f32, isOutput=False)
    bpd = nc.declare_dram_parameter("bp", [128, PC], f32, isOutput=False)
    outd = nc.declare_dram_parameter("out", [C, NQ], f32, isOutput=True)

    outr = outd.rearrange("(a p) n -> p a n", p=128)   # [128, 4, NQ]

    def r32(ap):
        return ap.bitcast(f32r)

    with tile.TileContext(nc) as tc:
        from contextlib import ExitStack

        with ExitStack() as ctx:
            consts = ctx.enter_context(tc.tile_pool(name="consts", bufs=1))
            kvq = ctx.enter_context(tc.tile_pool(name="kvq", bufs=1))

            ident = consts.tile([128, 128], f32)
            make_identity(nc, ident)
            ones_col = consts.tile([128, 1], f32r)
            nc.vector.memset(ones_col.bitcast(f32), 1.0)
            ones_row = consts.tile([1, 128], f32r)
            nc.vector.memset(ones_row.bitcast(f32), 1.0)
            ceps = consts.tile([128, 1], f32)
            nc.vector.memset(ceps, C * EPS)

            bq_sb = consts.tile([128, PC], f32)
            nc.gpsimd.dma_start(out=bq_sb, in_=bqd[:])
            bk_sb = consts.tile([128, PC], f32)
            nc.gpsimd.dma_start(out=bk_sb, in_=bkd[:])
            bp_sb = consts.tile([128, PC], f32)
            nc.gpsimd.dma_start(out=bp_sb, in_=bpd[:])

            wq_sb = consts.tile([128, PC, C], f32r)
            nc.gpsimd.dma_start(out=wq_sb, in_=wqd[:])
            wk_sb = consts.tile([128, PC, C], f32r)
            nc.gpsimd.dma_start(out=wk_sb, in_=wkd[:])
            wv_sb = consts.tile([128, PC, C], f32r)
            nc.gpsimd.dma_start(out=wv_sb, in_=wvd[:])
            wp_sb = consts.tile([128, PC, C], f32r)
            nc.gpsimd.dma_start(out=wp_sb, in_=wpd[:])

            k_all = kvq.tile([128, PC, N], f16)    # (c, n) layout
            v_all = kvq.tile([128, NKC, C], f16)   # (n, c) layout
            q_all = kvq.tile([128, PC, NQ], f16)   # (c, nq) layout

            # R_row: per-column rsqrt factor, exp(-0.5 ln(C*(var+eps)))
            r_row = consts.tile([1, N], f32r)

            # ------- phase 1: two passes. A: column stats (all chunks, one
            # Ln+Exp batch = one table set load). B: x'=x*r + K/V/Q matmuls.
            with ExitStack() as p1:
                stage = p1.enter_context(tc.tile_pool(name="stage", bufs=2))
                xa = p1.enter_context(tc.tile_pool(name="xa", bufs=3))
                x2p = p1.enter_context(tc.tile_pool(name="x2", bufs=2))
                xb = p1.enter_context(tc.tile_pool(name="xb", bufs=3))
                xpp = p1.enter_context(tc.tile_pool(name="xp", bufs=3))
                pstat = p1.enter_context(
                    tc.tile_pool(name="pstat", bufs=2, space=bass.MemorySpace.PSUM)
                )
                rrep = p1.enter_context(
                    tc.tile_pool(name="rrep", bufs=2, space=bass.MemorySpace.PSUM)
                )
                kvps = p1.enter_context(
                    tc.tile_pool(name="kvps", bufs=4, space=bass.MemorySpace.PSUM)
                )

                # --- pass A: stats, in halves so half-0's math chain overlaps
                # half-1's stats matmuls; x loads rotate across DMA queues ---
                HN = NCH // 2
                dmaeng = [nc.sync, nc.scalar, nc.gpsimd]
                for hh in range(2):
                    stg_row = stage.tile([1, HN * 2 * CH], f32, name="stg_row")
                    for jj in range(HN):
                        j = hh * HN + jj
                        xt = xa.tile([128, PC, CH], f32r, tag="xt", name="xt")
                        dmaeng[j % 3].dma_start(out=xt, in_=xd[j])
                        x2t = x2p.tile([128, PC, CH], f32r)
                        nc.vector.tensor_mul(x2t, xt, xt)
                        ps = pstat.tile([1, 2 * CH], f32)
                        for ci in range(PC):
                            nc.tensor.matmul(
                                ps[:, 0:CH], ones_col, xt[:, ci, :],
                                start=(ci == 0), stop=(ci == PC - 1),
                            )
                        for ci in range(PC):
                            nc.tensor.matmul(
                                ps[:, CH:2 * CH], ones_col, x2t[:, ci, :],
                                start=(ci == 0), stop=(ci == PC - 1),
                            )
                        nc.scalar.copy(
                            stg_row[0:1, jj * 2 * CH:(jj + 1) * 2 * CH], ps
                        )
                    stg = stage.tile([HN, 2 * CH], f32, name="stg")
                    nc.sync.dma_start(out=stg, in_=stg_row)
                    u2 = stage.tile([HN, CH], f32, name="u2")
                    nc.vector.tensor_mul(u2, stg[:, 0:CH], stg[:, 0:CH])
                    nc.vector.tensor_scalar_mul(u2, u2, 1.0 / C)
                    w1 = stage.tile([HN, CH], f32, name="w1")
                    nc.vector.tensor_tensor(
                        out=w1, in0=stg[:, CH:2 * CH], in1=u2, op=ALU.subtract
                    )
                    nc.scalar.activation(w1, w1, AF.Ln, bias=ceps[0:HN])
                    rt = stage.tile([HN, CH], f32r, name="rt")
                    nc.scalar.activation(rt, w1, AF.Exp, scale=-0.5)
                    nc.sync.dma_start(
                        out=r_row[0:1, hh * HN * CH:(hh + 1) * HN * CH], in_=rt
                    )

                # --- pass B: x', K, V, Q ---
                for j2 in range(NCH // 2):  # 512-col pairs
                    xps = []
                    for h in range(2):
                        j = 2 * j2 + h
                        xt = xb.tile([128, PC, CH], f32r, name="xtb")
                        dmaeng[j % 3].dma_start(out=xt, in_=xd[j])
                        rr = rrep.tile([128, CH], f32)
                        nc.tensor.matmul(
                            rr, ones_row,
                            r_row[0:1, j * CH:(j + 1) * CH],
                            start=True, stop=True,
                        )
                        xpt = xpp.tile([128, PC, CH], f32r)
                        nc.vector.tensor_mul(
                            xpt, xt, rr.unsqueeze(1).broadcast_to([128, PC, CH])
                        )
                        xps.append(xpt)

                    for co in range(PC):
                        kp = kvps.tile([128, 2 * CH], f32, tag="kvqps", name="kp")
                        for h in range(2):
                            for ci in range(PC):
                                nc.tensor.matmul(
                                    kp[:, h * CH:(h + 1) * CH],
                                    wk_sb[:, ci, co * 128:(co + 1) * 128],
                                    xps[h][:, ci, :],
                                    start=(ci == 0), stop=(ci == PC - 1),
                                )
                        nc.scalar.activation(
                            k_all[:, co, j2 * 512:(j2 + 1) * 512], kp,
                            AF.Identity, bias=bk_sb[:, co:co + 1],
                        )
                    for h in range(2):
                        for s in range(2):
                            vp = kvps.tile([128, C], f32, tag="kvqps", name="vp")
                            for ci in range(PC):
                                nc.tensor.matmul(
                                    vp,
                                    xps[h][:, ci, s * 128:(s + 1) * 128],
                                    wv_sb[:, ci, :],
                                    start=(ci == 0), stop=(ci == PC - 1),
                                )
                            nc.vector.tensor_copy(
                                v_all[:, 4 * j2 + 2 * h + s, :], vp
                            )
                    if j2 < 2:
                        for co in range(PC):
                            qp = kvps.tile([128, 2 * CH], f32, tag="kvqps", name="qp")
                            for h in range(2):
                                for ci in range(PC):
                                    nc.tensor.matmul(
                                        qp[:, h * CH:(h + 1) * CH],
                                        wq_sb[:, ci, co * 128:(co + 1) * 128],
                                        xps[h][:, ci, :],
                                        start=(ci == 0), stop=(ci == PC - 1),
                                    )
                            nc.scalar.activation(
                                q_all[:, co, j2 * 512:(j2 + 1) * 512], qp,
                                AF.Identity, bias=bq_sb[:, co:co + 1],
                            )

            # ---------------- attention + projection, per 512-query group ----------------
            with ExitStack() as pat:
                stp = pat.enter_context(
                    tc.tile_pool(name="stp", bufs=3, space=bass.MemorySpace.PSUM)
                )
                avp_pool = pat.enter_context(
                    tc.tile_pool(name="avp", bufs=4, space=bass.MemorySpace.PSUM)
                )
                tp_pool = pat.enter_context(
                    tc.tile_pool(name="tp", bufs=1, space=bass.MemorySpace.PSUM)
                )
                ptp = pat.enter_context(tc.tile_pool(name="ptp", bufs=4))
                avn_pool = pat.enter_context(tc.tile_pool(name="avn", bufs=4))
                avt_pool = pat.enter_context(tc.tile_pool(name="avt", bufs=2))
                out_pool = pat.enter_context(tc.tile_pool(name="outp", bufs=2))
                xres_pool = pat.enter_context(tc.tile_pool(name="xres", bufs=4))
                small = pat.enter_context(tc.tile_pool(name="small", bufs=2))

                # prefetch residual inputs for both groups (per 256-col chunk)
                xres_ts = []
                for g in range(2):
                    row = []
                    for h in range(2):
                        xres = xres_pool.tile(
                            [128, PC, CH], f32r, tag="xres", name=f"xres{g}{h}"
                        )
                        (nc.sync if h == 0 else nc.scalar).dma_start(
                            out=xres, in_=xd[2 * g + h]
                        )
                        row.append(xres)
                    xres_ts.append(row)

                for g in range(2):
                    q0 = g * 512
                    avps = [avp_pool.tile([128, C], f32, tag="av", name=f"avp{s}") for s in range(4)]
                    # sumexp accumulated on DVE (two chains), reduced by one matmul
                    acc0 = small.tile([128, 512], f32r, tag="acc", name="acc0")
                    acc1 = small.tile([128, 512], f32r, tag="acc", name="acc1")
                    for jk in range(NKC):
                        st = stp.tile([128, 512], f32)
                        for ci in range(PC):
                            nc.tensor.matmul(
                                st,
                                k_all[:, ci, jk * 128:(jk + 1) * 128],
                                q_all[:, ci, q0:q0 + 512],
                                start=(ci == 0), stop=(ci == PC - 1),
                            )
                        pt = ptp.tile([128, 512], f16)
                        nc.scalar.activation(pt, st, AF.Exp)
                        acc = acc0 if jk % 2 == 0 else acc1
                        if jk < 2:
                            nc.vector.tensor_copy(acc, pt)
                        else:
                            nc.vector.tensor_add(acc, acc, pt)
                        for s in range(4):
                            nc.tensor.matmul(
                                avps[s],
                                pt[:, s * 128:(s + 1) * 128],
                                v_all[:, jk, :],
                                start=(jk == 0), stop=(jk == NKC - 1),
                            )
                    nc.vector.tensor_add(acc0, acc0, acc1)
                    sep = tp_pool.tile([1, 512], f32, tag="tp", name="sep")
                    nc.tensor.matmul(sep, ones_col, acc0, start=True, stop=True)

                    # 1/sumexp as per-partition columns
                    serow = small.tile([1, 512], f32)
                    nc.scalar.copy(serow, sep)
                    rc_ps = tp_pool.tile([128, 512], f32, tag="tp")
                    for s in range(4):
                        nc.tensor.transpose(
                            rc_ps[:, s:s + 1],
                            serow[0:1, s * 128:(s + 1) * 128],
                            ident[0:1, 0:1],
                        )
                    rc_sb = small.tile([128, 4], f32)
                    nc.vector.reciprocal(rc_sb, rc_ps[:, 0:4])

                    # normalize + transpose to (c, nq)
                    avns = []
                    for s in range(4):
                        avn = avn_pool.tile([128, C], f32, tag="avn", name=f"avn{s}")
                        nc.vector.tensor_scalar_mul(avn, avps[s], rc_sb[:, s:s + 1])
                        avns.append(avn)
                    avt = avt_pool.tile([128, PC, 512], f32r)
                    for ci in range(PC):
                        tps = tp_pool.tile([128, 512], f32, tag="tp")
                        for s in range(4):
                            nc.tensor.transpose(
                                tps[:, s * 128:(s + 1) * 128],
                                avns[s][:, ci * 128:(ci + 1) * 128],
                                ident,
                            )
                        nc.vector.tensor_copy(avt[:, ci, :], tps)

                    # projection + bias + residual + store
                    out_t = out_pool.tile([128, PC, 512], f32)
                    for co in range(PC):
                        pop = tp_pool.tile([128, 512], f32, tag="tp")
                        for ci in range(PC):
                            nc.tensor.matmul(
                                pop,
                                wp_sb[:, ci, co * 128:(co + 1) * 128],
                                avt[:, ci, :],
                                start=(ci == 0), stop=(ci == PC - 1),
                            )
                        nc.scalar.activation(
                            out_t[:, co, :], pop,
                            AF.Identity, bias=bp_sb[:, co:co + 1],
                        )
                        for h in range(2):
                            nc.vector.tensor_add(
                                out_t[:, co, h * CH:(h + 1) * CH],
                                out_t[:, co, h * CH:(h + 1) * CH],
                                xres_ts[g][h][:, co, :],
                            )
                        nc.sync.dma_start(
                            out=outr[:, co, q0:q0 + 512], in_=out_t[:, co, :]
                        )


    nc.compile()
    return nc


def _get_nc():
    global _cached_nc
    if _cached_nc is None:
        _cached_nc = _build_nc()
    return _cached_nc


def kernel(x, norm_w, w_qkv, b_qkv, w_proj, b_proj):
    x = np.asarray(x, dtype=np.float32)
    norm_w = np.asarray(norm_w, dtype=np.float32)
    w_qkv = np.asarray(w_qkv, dtype=np.float32)
    b_qkv = np.asarray(b_qkv, dtype=np.float32)
    w_proj = np.asarray(w_proj, dtype=np.float32)
    b_proj = np.asarray(b_proj, dtype=np.float32)

    B = x.shape[0]
    scale = C ** -0.5
    sc = np.sqrt(C).astype(np.float32)

    # fold norm_w + LN centering + sqrt(C) (+ attention scale for q) into weights
    Wq = w_qkv[0:C] * norm_w[None, :]
    Wk = w_qkv[C:2 * C] * norm_w[None, :]
    Wv = w_qkv[2 * C:3 * C] * norm_w[None, :]
    def wtile(wt):  # [cin, cout] -> [128, PC, cout]
        return np.ascontiguousarray(wt.reshape(PC, 128, C).transpose(1, 0, 2))

    Wqt = wtile(((Wq - Wq.mean(1, keepdims=True)) * (sc * scale)).T)
    Wkt = wtile(((Wk - Wk.mean(1, keepdims=True)) * sc).T)
    Wvt = wtile(((Wv - Wv.mean(1, keepdims=True)) * sc).T)
    Wpt = wtile(w_proj.T)

    def cols(b):  # [C] -> [128, 4] chunk-column layout
        return np.ascontiguousarray(b.reshape(PC, 128).T)

    bq = cols(b_qkv[0:C] * scale)
    bk = cols(b_qkv[C:2 * C])
    bv = b_qkv[2 * C:3 * C]
    bpt = cols(b_proj + w_proj @ bv)

    in_maps = []
    for core in range(8):
        bi, qi = core // 4, core % 4
        xl = np.roll(x[bi].reshape(C, N), -qi * NQ, axis=1)
        # pre-tile to the on-chip layout: [chunk, partition, c-chunk, col]
        xl = np.ascontiguousarray(
            xl.reshape(PC, 128, NCH, CH).transpose(2, 1, 0, 3)
        )
        in_maps.append({
            "x": xl, "wq": Wqt, "wk": Wkt, "wv": Wvt, "wp": Wpt,
            "bq": bq, "bk": bk, "bp": bpt,
        })

    from concourse.bass_utils import run_bass_kernel_spmd

    nc = _get_nc()
    res = run_bass_kernel_spmd(nc, in_maps, core_ids=list(range(8)))

    out = np.empty((B, C, N), dtype=np.float32)
    for core in range(8):
        bi, qi = core // 4, core % 4
        out[bi][:, qi * NQ:(qi + 1) * NQ] = res.results[core]["out"]
    return out.reshape(x.shape)


# revision 19
# speedup vs baseline: 1.3144x; 1.0020x over previous
"""AttentionBlock (b=2, c=512, 64x64) on 8 trn2 NeuronCores.

Sharding: core i handles batch i//4, query rows (i%4)*1024..+1024 (of the
4096 flattened h*w positions). Each core receives its batch's full x with
columns rotated so its own query block sits at columns 0:1024, computes
LayerNorm + K + V for all 4096 positions (replicated inside the 4-core
batch group) and Q/attention/projection for its 1024 queries.

Math reformulation (validated to ~1e-7 against the jax reference):
  - norm_w and the LayerNorm centering are folded into the QKV weights on
    the host: W~ = (W*norm_w) - row_mean(W*norm_w); then
    qkv = rsqrt(var+eps) * (W~ @ x) + b  -- no on-device mean subtraction.
  - rsqrt(var+eps) = sqrt(C) * exp(-0.5*ln(Sx2 - Sx^2/C + C*eps)); the
    sqrt(C) constant and the attention scale C**-0.5 are folded into the
    host weights, so the device only needs ln/exp (one ACT table set).
  - scores are computed transposed, sT[nk,nq] = k^T q; softmax skips the
    max-subtraction (logits are bounded ~ +-6) and defers normalization:
    av_raw = exp(sT)^T V, sumexp via a ones-vector matmul, divide at PSUM
    eviction time. The V bias is folded into the proj bias on the host.
All matmuls run as float32r (full PE speed); K/V/Q/exp(s) are stored fp16.
"""
import sys

if "/opt/trn_rl_repo" not in sys.path:
    sys.path.insert(0, "/opt/trn_rl_repo")

import numpy as np

C = 512          # channels
N = 4096         # h*w positions
NQ = 1024        # queries per core
PC = 4           # c chunks of 128
NKC = 32         # key chunks of 128
NCH = 16         # phase-1 column chunks of 256
CH = 256         # phase-1 streaming chunk width
EPS = 1e-5

_cached_nc = None


def _build_nc():
    import concourse.bass as bass
    import concourse.tile as tile
    from concourse import bacc, mybir
    from concourse.masks import make_identity

    f32 = mybir.dt.float32
    f32r = mybir.dt.float32r
    f16 = mybir.dt.float16
    AF = mybir.ActivationFunctionType
    ALU = mybir.AluOpType

    nc = bacc.Bacc(None, target_bir_lowering=False)

    xd = nc.declare_dram_parameter("x", [NCH, 128, PC, CH], f32r, isOutput=False)
    wqd = nc.declare_dram_parameter("wq", [128, PC, C], f32r, isOutput=False)
    wkd = nc.declare_dram_parameter("wk", [128, PC, C], f32r, isOutput=False)
    wvd = nc.declare_dram_parameter("wv", [128, PC, C], f32r, isOutput=False)
    wpd = nc.declare_dram_parameter("wp", [128, PC, C], f32r, isOutput=False)
    bqd = nc.declare_dram_parameter("bq", [128, PC], f32, isOutput=False)
    bkd = nc.declare_dram_parameter("bk", [128, PC], f32, isOutput=False)
    bpd = nc.declare_dram_parameter("bp", [128, PC], f32, isOutput=False)
    outd = nc.declare_dram_parameter("out", [C, NQ], f32, isOutput=True)

    outr = outd.rearrange("(a p) n -> p a n", p=128)   # [128, 4, NQ]

    def r32(ap):
        return ap.bitcast(f32r)

    with tile.TileContext(nc) as tc:
        from contextlib import ExitStack

        with ExitStack() as ctx:
            consts = ctx.enter_context(tc.tile_pool(name="consts", bufs=1))
            kvq = ctx.enter_context(tc.tile_pool(name="kvq", bufs=1))

            ident = consts.tile([128, 128], f32)
            make_identity(nc, ident)
            ones_col = consts.tile([128, 1], f32r)
            nc.vector.memset(ones_col.bitcast(f32), 1.0)
            ones_row = consts.tile([1, 128], f32r)
            nc.vector.memset(ones_row.bitcast(f32), 1.0)
            ceps = consts.tile([128, 1], f32)
            nc.vector.memset(ceps, C * EPS)

            bq_sb = consts.tile([128, PC], f32)
            bk_sb = consts.tile([128, PC], f32)
            bp_sb = consts.tile([128, PC], f32)
            wq_sb = consts.tile([128, PC, C], f32r)
            wk_sb = consts.tile([128, PC, C], f32r)
            wv_sb = consts.tile([128, PC, C], f32r)
            wp_sb = consts.tile([128, PC, C], f32r)

            k_all = kvq.tile([128, PC, N], f16)    # (c, n) layout
            v_all = kvq.tile([128, NKC, C], f16)   # (n, c) layout
            q_all = kvq.tile([128, PC, NQ], f16)   # (c, nq) layout

            # R_row: per-column rsqrt factor, exp(-0.5 ln(C*(var+eps)))
            r_row = consts.tile([1, N], f32r)

            # ------- phase 1: two passes. A: column stats (all chunks, one
            # Ln+Exp batch = one table set load). B: x'=x*r + K/V/Q matmuls.
            with ExitStack() as p1:
                stage = p1.enter_context(tc.tile_pool(name="stage", bufs=2))
                xa = p1.enter_context(tc.tile_pool(name="xa", bufs=3))
                x2p = p1.enter_context(tc.tile_pool(name="x2", bufs=2))
                xb = p1.enter_context(tc.tile_pool(name="xb", bufs=3))
                xpp = p1.enter_context(tc.tile_pool(name="xp", bufs=3))
                pstat = p1.enter_context(
                    tc.tile_pool(name="pstat", bufs=2, space=bass.MemorySpace.PSUM)
                )
                rrep = p1.enter_context(
                    tc.tile_pool(name="rrep", bufs=2, space=bass.MemorySpace.PSUM)
                )
                kvps = p1.enter_context(
                    tc.tile_pool(name="kvps", bufs=4, space=bass.MemorySpace.PSUM)
                )

                # --- pass A: stats, in halves so half-0's math chain overlaps
                # half-1's stats matmuls; x loads rotate across DMA queues ---
                HN = NCH // 2
                dmaeng = [nc.sync, nc.scalar, nc.gpsimd]
                for hh in range(2):
                    stg_row = stage.tile([1, HN * 2 * CH], f32, name="stg_row")
                    for jj in range(HN):
                        j = hh * HN + jj
                        xt = xa.tile([128, PC, CH], f32r, tag="xt", name="xt")
                        dmaeng[j % 3].dma_start(out=xt, in_=xd[j])
                        x2t = x2p.tile([128, PC, CH], f32r)
                        nc.vector.tensor_mul(x2t, xt, xt)
                        ps = pstat.tile([1, 2 * CH], f32)
                        for ci in range(PC):
                            nc.tensor.matmul(
                                ps[:, 0:CH], ones_col, xt[:, ci, :],
                                start=(ci == 0), stop=(ci == PC - 1),
                            )
                        for ci in range(PC):
                            nc.tensor.matmul(
                                ps[:, CH:2 * CH], ones_col, x2t[:, ci, :],
                                start=(ci == 0), stop=(ci == PC - 1),
                            )
                        nc.scalar.copy(
                            stg_row[0:1, jj * 2 * CH:(jj + 1) * 2 * CH], ps
                        )
                    if hh == 0:
                        # weights/biases issued behind the first half's x
                        # loads: they transfer while stats matmuls run and
                        # are resident before the first K matmul needs them
                        nc.sync.dma_start(out=bk_sb, in_=bkd[:])
                        nc.scalar.dma_start(out=bq_sb, in_=bqd[:])
                        nc.gpsimd.dma_start(out=bp_sb, in_=bpd[:])
                        nc.sync.dma_start(out=wk_sb, in_=wkd[:])
                        nc.scalar.dma_start(out=wv_sb, in_=wvd[:])
                        nc.gpsimd.dma_start(out=wq_sb, in_=wqd[:])
                        nc.sync.dma_start(out=wp_sb, in_=wpd[:])
                    stg = stage.tile([HN, 2 * CH], f32, name="stg")
                    nc.sync.dma_start(out=stg, in_=stg_row)
                    u2 = stage.tile([HN, CH], f32, name="u2")
                    nc.vector.tensor_mul(u2, stg[:, 0:CH], stg[:, 0:CH])
                    nc.vector.tensor_scalar_mul(u2, u2, 1.0 / C)
                    w1 = stage.tile([HN, CH], f32, name="w1")
                    nc.vector.tensor_tensor(
                        out=w1, in0=stg[:, CH:2 * CH], in1=u2, op=ALU.subtract
                    )
                    nc.scalar.activation(w1, w1, AF.Ln, bias=ceps[0:HN])
                    rt = stage.tile([HN, CH], f32r, name="rt")
                    nc.scalar.activation(rt, w1, AF.Exp, scale=-0.5)
                    nc.sync.dma_start(
                        out=r_row[0:1, hh * HN * CH:(hh + 1) * HN * CH], in_=rt
                    )

                # --- pass B: x', K, V, Q ---
                for j2 in range(NCH // 2):  # 512-col pairs
                    xps = []
                    for h in range(2):
                        j = 2 * j2 + h
                        xt = xb.tile([128, PC, CH], f32r, name="xtb")
                        dmaeng[j % 3].dma_start(out=xt, in_=xd[j])
                        rr = rrep.tile([128, CH], f32)
                        nc.tensor.matmul(
                            rr, ones_row,
                            r_row[0:1, j * CH:(j + 1) * CH],
                            start=True, stop=True,
                        )
                        xpt = xpp.tile([128, PC, CH], f32r)
                        nc.vector.tensor_mul(
                            xpt, xt, rr.unsqueeze(1).broadcast_to([128, PC, CH])
                        )
                        xps.append(xpt)

                    for co in range(PC):
                        kp = kvps.tile([128, 2 * CH], f32, tag="kvqps", name="kp")
                        for h in range(2):
                            for ci in range(PC):
                                nc.tensor.matmul(
                                    kp[:, h * CH:(h + 1) * CH],
                                    wk_sb[:, ci, co * 128:(co + 1) * 128],
                                    xps[h][:, ci, :],
                                    start=(ci == 0), stop=(ci == PC - 1),
                                )
                        nc.scalar.activation(
                            k_all[:, co, j2 * 512:(j2 + 1) * 512], kp,
                            AF.Identity, bias=bk_sb[:, co:co + 1],
                        )
                    for h in range(2):
                        for s in range(2):
                            vp = kvps.tile([128, C], f32, tag="kvqps", name="vp")
                            for ci in range(PC):
                                nc.tensor.matmul(
                                    vp,
                                    xps[h][:, ci, s * 128:(s + 1) * 128],
                                    wv_sb[:, ci, :],
                                    start=(ci == 0), stop=(ci == PC - 1),
                                )
                            nc.vector.tensor_copy(
                                v_all[:, 4 * j2 + 2 * h + s, :], vp
                            )
                    if j2 < 2:
                        for co in range(PC):
                            qp = kvps.tile([128, 2 * CH], f32, tag="kvqps", name="qp")
                            for h in range(2):
                                for ci in range(PC):
                                    nc.tensor.matmul(
                                        qp[:, h * CH:(h + 1) * CH],
                                        wq_sb[:, ci, co * 128:(co + 1) * 128],
                                        xps[h][:, ci, :],
                                        start=(ci == 0), stop=(ci == PC - 1),
                                    )
                            nc.scalar.activation(
                                q_all[:, co, j2 * 512:(j2 + 1) * 512], qp,
                                AF.Identity, bias=bq_sb[:, co:co + 1],
                            )

            # ---------------- attention + projection, per 512-query group ----------------
            with ExitStack() as pat:
                stp = pat.enter_context(
                    tc.tile_pool(name="stp", bufs=3, space=bass.MemorySpace.PSUM)
                )
                avp_pool = pat.enter_context(
                    tc.tile_pool(name="avp", bufs=4, space=bass.MemorySpace.PSUM)
                )
                tp_pool = pat.enter_context(
                    tc.tile_pool(name="tp", bufs=1, space=bass.MemorySpace.PSUM)
                )
                ptp = pat.enter_context(tc.tile_pool(name="ptp", bufs=4))
                avn_pool = pat.enter_context(tc.tile_pool(name="avn", bufs=4))
                avt_pool = pat.enter_context(tc.tile_pool(name="avt", bufs=2))
                out_pool = pat.enter_context(tc.tile_pool(name="outp", bufs=2))
                xres_pool = pat.enter_context(tc.tile_pool(name="xres", bufs=4))
                small = pat.enter_context(tc.tile_pool(name="small", bufs=2))

                # prefetch residual inputs for both groups (per 256-col chunk)
                xres_ts = []
                for g in range(2):
                    row = []
                    for h in range(2):
                        xres = xres_pool.tile(
                            [128, PC, CH], f32r, tag="xres", name=f"xres{g}{h}"
                        )
                        (nc.sync if h == 0 else nc.scalar).dma_start(
                            out=xres, in_=xd[2 * g + h]
                        )
                        row.append(xres)
                    xres_ts.append(row)

                for g in range(2):
                    q0 = g * 512
                    avps = [avp_pool.tile([128, C], f32, tag="av", name=f"avp{s}") for s in range(4)]
                    # sumexp accumulated on DVE (two chains), reduced by one matmul
                    acc0 = small.tile([128, 512], f32r, tag="acc", name="acc0")
                    acc1 = small.tile([128, 512], f32r, tag="acc", name="acc1")
                    for jk in range(NKC):
                        st = stp.tile([128, 512], f32)
                        for ci in range(PC):
                            nc.tensor.matmul(
                                st,
                                k_all[:, ci, jk * 128:(jk + 1) * 128],
                                q_all[:, ci, q0:q0 + 512],
                                start=(ci == 0), stop=(ci == PC - 1),
                            )
                        pt = ptp.tile([128, 512], f16)
                        nc.scalar.activation(pt, st, AF.Exp)
                        acc = acc0 if jk % 2 == 0 else acc1
                        if jk < 2:
                            nc.vector.tensor_copy(acc, pt)
                        else:
                            nc.vector.tensor_add(acc, acc, pt)
                        for s in range(4):
                            nc.tensor.matmul(
                                avps[s],
                                pt[:, s * 128:(s + 1) * 128],
                                v_all[:, jk, :],
                                start=(jk == 0), stop=(jk == NKC - 1),
                            )
                    nc.vector.tensor_add(acc0, acc0, acc1)
                    sep = tp_pool.tile([1, 512], f32, tag="tp", name="sep")
                    nc.tensor.matmul(sep, ones_col, acc0, start=True, stop=True)

                    # 1/sumexp as per-partition columns
                    serow = small.tile([1, 512], f32)
                    nc.scalar.copy(serow, sep)
                    rc_ps = tp_pool.tile([128, 512], f32, tag="tp")
                    for s in range(4):
                        nc.tensor.transpose(
                            rc_ps[:, s:s + 1],
                            serow[0:1, s * 128:(s + 1) * 128],
                            ident[0:1, 0:1],
                        )
                    rc_sb = small.tile([128, 4], f32)
                    nc.vector.reciprocal(rc_sb, rc_ps[:, 0:4])

                    # normalize + transpose to (c, nq)
                    avns = []
                    for s in range(4):
                        avn = avn_pool.tile([128, C], f32, tag="avn", name=f"avn{s}")
                        nc.vector.tensor_scalar_mul(avn, avps[s], rc_sb[:, s:s + 1])
                        avns.append(avn)
                    avt = avt_pool.tile([128, PC, 512], f32r)
                    for ci in range(PC):
                        tps = tp_pool.tile([128, 512], f32, tag="tp")
                        for s in range(4):
                            nc.tensor.transpose(
                                tps[:, s * 128:(s + 1) * 128],
                                avns[s][:, ci * 128:(ci + 1) * 128],
                                ident,
                            )
                        nc.vector.tensor_copy(avt[:, ci, :], tps)

                    # projection + bias + residual + store
                    out_t = out_pool.tile([128, PC, 512], f32)
                    for co in range(PC):
                        pop = tp_pool.tile([128, 512], f32, tag="tp")
                        for ci in range(PC):
                            nc.tensor.matmul(
                                pop,
                                wp_sb[:, ci, co * 128:(co + 1) * 128],
                                avt[:, ci, :],
                                start=(ci == 0), stop=(ci == PC - 1),
                            )
                        nc.scalar.activation(
                            out_t[:, co, :], pop,
                            AF.Identity, bias=bp_sb[:, co:co + 1],
                        )
                        for h in range(2):
                            nc.vector.tensor_add(
                                out_t[:, co, h * CH:(h + 1) * CH],
                                out_t[:, co, h * CH:(h + 1) * CH],
                                xres_ts[g][h][:, co, :],
                            )
                        nc.sync.dma_start(
                            out=outr[:, co, q0:q0 + 512], in_=out_t[:, co, :]
                        )


    nc.compile()
    return nc


def _get_nc():
    global _cached_nc
    if _cached_nc is None:
        _cached_nc = _build_nc()
    return _cached_nc


def kernel(x, norm_w, w_qkv, b_qkv, w_proj, b_proj):
    x = np.asarray(x, dtype=np.float32)
    norm_w = np.asarray(norm_w, dtype=np.float32)
    w_qkv = np.asarray(w_qkv, dtype=np.float32)
    b_qkv = np.asarray(b_qkv, dtype=np.float32)
    w_proj = np.asarray(w_proj, dtype=np.float32)
    b_proj = np.asarray(b_proj, dtype=np.float32)

    B = x.shape[0]
    scale = C ** -0.5
    sc = np.sqrt(C).astype(np.float32)

    # fold norm_w + LN centering + sqrt(C) (+ attention scale for q) into weights
    Wq = w_qkv[0:C] * norm_w[None, :]
    Wk = w_qkv[C:2 * C] * norm_w[None, :]
    Wv = w_qkv[2 * C:3 * C] * norm_w[None, :]
    def wtile(wt):  # [cin, cout] -> [128, PC, cout]
        return np.ascontiguousarray(wt.reshape(PC, 128, C).transpose(1, 0, 2))

    Wqt = wtile(((Wq - Wq.mean(1, keepdims=True)) * (sc * scale)).T)
    Wkt = wtile(((Wk - Wk.mean(1, keepdims=True)) * sc).T)
    Wvt = wtile(((Wv - Wv.mean(1, keepdims=True)) * sc).T)
    Wpt = wtile(w_proj.T)

    def cols(b):  # [C] -> [128, 4] chunk-column layout
        return np.ascontiguousarray(b.reshape(PC, 128).T)

    bq = cols(b_qkv[0:C] * scale)
    bk = cols(b_qkv[C:2 * C])
    bv = b_qkv[2 * C:3 * C]
    bpt = cols(b_proj + w_proj @ bv)

    in_maps = []
    for core in range(8):
        bi, qi = core // 4, core % 4
        xl = np.roll(x[bi].reshape(C, N), -qi * NQ, axis=1)
        # pre-tile to the on-chip layout: [chunk, partition, c-chunk, col]
        xl = np.ascontiguousarray(
            xl.reshape(PC, 128, NCH, CH).transpose(2, 1, 0, 3)
        )
        in_maps.append({
            "x": xl, "wq": Wqt, "wk": Wkt, "wv": Wvt, "wp": Wpt,
            "bq": bq, "bk": bk, "bp": bpt,
        })

    from concourse.bass_utils import run_bass_kernel_spmd

    nc = _get_nc()
    res = run_bass_kernel_spmd(nc, in_maps, core_ids=list(range(8)))

    out = np.empty((B, C, N), dtype=np.float32)
    for core in range(8):
        bi, qi = core // 4, core % 4
        out[bi][:, qi * NQ:(qi + 1) * NQ] = res.results[core]["out"]
    return out.reshape(x.shape)
